# revision 8
# baseline (speedup 1.0000x reference)
"""Trainium2 Bass kernel for the 2-layer LSTM decoder (7-day unroll).

Sharding: data-parallel over batch (4096 -> 512 rows/core on 8 cores),
weights replicated. All on-device tensors are stored transposed
([feature, batch]) so no on-device transposes are needed; host numpy
does the (free) transposes and gathers.

Math notes:
  - gates = Wih @ x + Whh @ h + b, with x = W_in @ cur + b_in folded in:
      W_eff = Wih0 @ W_in.T  (2048x4),  b_eff0 = bih0 + bhh0 + Wih0 @ b_in
    so layer0 gates = W_eff @ cur + Whh0 @ h0 + b_eff0 (cur is 4-dim).
  - W_out rows are permuted on host to [mu(4) | log_sigma(4)] so the
    device sees clean partition slices.
  - Matmuls run in float32r (TF32-like, full PE rate at N=512).
"""
import numpy as np

import concourse.bass as bass
import concourse.mybir as mybir
import concourse.tile as tile
from concourse import bacc
from concourse.bass_utils import run_bass_kernel_spmd

HIDDEN = 512
BATCH = 4096
DAYS = 7
NTGT = 4
NIN = 4
NCORES = 8
BLOC = BATCH // NCORES          # 512 batch rows per core
KT = HIDDEN // 128              # 4 contraction tiles
GT = 4 * HIDDEN // 128          # 16 gate tiles
F32 = mybir.dt.float32
F32R = mybir.dt.float32r
import os
if os.environ.get("KMM_DTYPE") == "f32":
    F32R = mybir.dt.float32
AF = mybir.ActivationFunctionType
ALU = mybir.AluOpType

_CACHE = {}


def _lstm_layer(nc, pspool, gpool, tpool, mm_specs, bias_sb, c_sb, h_new_sb):
    """One LSTM cell update in transposed layout.

    mm_specs: list of (lhsT_fn, rhs) pairs; lhsT_fn(gate_slice) -> AP.
    Gates accumulate in PSUM per 128-row chunk j; ScalarE applies
    bias+nonlinearity straight out of PSUM; VectorE does the cell update.
    """
    for j in range(KT):
        ps = pspool.tile([128, 4, BLOC], F32, tag="ps")
        for g in range(4):  # i, f, g, o
            m = g * KT + j
            sl = slice(m * 128, (m + 1) * 128)
            n_mm = len(mm_specs)
            for idx, (lhsT_fn, rhs) in enumerate(mm_specs):
                nc.tensor.matmul(
                    ps[:, g, :], lhsT_fn(sl), rhs,
                    start=(idx == 0), stop=(idx == n_mm - 1),
                )
        gi = gpool.tile([128, BLOC], F32, tag="gi")
        gf = gpool.tile([128, BLOC], F32, tag="gf")
        gg = gpool.tile([128, BLOC], F32, tag="gg")
        go = gpool.tile([128, BLOC], F32, tag="go")
        nc.scalar.activation(gi[:], ps[:, 0, :], AF.Sigmoid,
                             bias=bias_sb[:, 0 * KT + j: 0 * KT + j + 1])
        nc.scalar.activation(gf[:], ps[:, 1, :], AF.Sigmoid,
                             bias=bias_sb[:, 1 * KT + j: 1 * KT + j + 1])
        nc.scalar.activation(gg[:], ps[:, 2, :], AF.Tanh,
                             bias=bias_sb[:, 2 * KT + j: 2 * KT + j + 1])
        nc.scalar.activation(go[:], ps[:, 3, :], AF.Sigmoid,
                             bias=bias_sb[:, 3 * KT + j: 3 * KT + j + 1])
        t1 = tpool.tile([128, BLOC], F32, tag="t1")
        t2 = tpool.tile([128, BLOC], F32, tag="t2")
        th = tpool.tile([128, BLOC], F32, tag="th")
        nc.vector.tensor_mul(t1[:], gi[:], gg[:])
        nc.vector.tensor_mul(t2[:], gf[:], c_sb[:, j, :])
        nc.vector.tensor_add(c_sb[:, j, :], t1[:], t2[:])
        nc.scalar.activation(th[:], c_sb[:, j, :], AF.Tanh)
        nc.vector.tensor_mul(h_new_sb[:, j, :], go[:], th[:])


def build_nc():
    nc = bacc.Bacc("TRN2", target_bir_lowering=False, debug=False,
                   num_devices=NCORES)
    G4 = 4 * HIDDEN

    weff0 = nc.declare_dram_parameter("weff0", [NIN, G4], F32R, isOutput=False)
    whh0 = nc.declare_dram_parameter("whh0", [HIDDEN, G4], F32R, isOutput=False)
    wih1 = nc.declare_dram_parameter("wih1", [HIDDEN, G4], F32R, isOutput=False)
    whh1 = nc.declare_dram_parameter("whh1", [HIDDEN, G4], F32R, isOutput=False)
    wout = nc.declare_dram_parameter("wout", [HIDDEN, 8], F32R, isOutput=False)
    wp = nc.declare_dram_parameter("wp", [HIDDEN, 1], F32R, isOutput=False)
    bg0 = nc.declare_dram_parameter("bg0", [128, GT], F32, isOutput=False)
    bg1 = nc.declare_dram_parameter("bg1", [128, GT], F32, isOutput=False)
    bout = nc.declare_dram_parameter("bout", [8, 1], F32, isOutput=False)
    bp = nc.declare_dram_parameter("bp", [1, 1], F32, isOutput=False)
    cur0 = nc.declare_dram_parameter("cur0", [NIN, BLOC], F32R, isOutput=False)
    h0 = nc.declare_dram_parameter("h0", [HIDDEN, BLOC], F32R, isOutput=False)
    c0 = nc.declare_dram_parameter("c0", [HIDDEN, BLOC], F32, isOutput=False)
    h1 = nc.declare_dram_parameter("h1", [HIDDEN, BLOC], F32R, isOutput=False)
    c1 = nc.declare_dram_parameter("c1", [HIDDEN, BLOC], F32, isOutput=False)
    mu_out = nc.declare_dram_parameter("mu_out", [DAYS, NTGT, BLOC], F32, isOutput=True)
    sg_out = nc.declare_dram_parameter("sg_out", [DAYS, NTGT, BLOC], F32, isOutput=True)
    p_out = nc.declare_dram_parameter("p_out", [DAYS, 1, BLOC], F32, isOutput=True)

    with tile.TileContext(nc) as tc:
        with (
            tc.tile_pool(name="wpool", bufs=1) as wpool,
            tc.tile_pool(name="state", bufs=1) as state,
            tc.tile_pool(name="bias", bufs=1) as bpool,
            tc.tile_pool(name="gpool", bufs=2) as gpool,
            tc.tile_pool(name="tpool", bufs=2) as tpool,
            tc.tile_pool(name="pspool", bufs=2, space="PSUM") as pspool,
        ):
            # ---- resident weights ----
            weff0_sb = wpool.tile([NIN, G4], F32R, tag="weff0")
            whh0_sb = wpool.tile([128, KT, G4], F32R, tag="whh0")
            wih1_sb = wpool.tile([128, KT, G4], F32R, tag="wih1")
            whh1_sb = wpool.tile([128, KT, G4], F32R, tag="whh1")
            wout_sb = wpool.tile([128, KT, 8], F32R, tag="wout")
            wp_sb = wpool.tile([128, KT, 1], F32R, tag="wp")
            bg0_sb = bpool.tile([128, GT], F32, tag="bg0")
            bg1_sb = bpool.tile([128, GT], F32, tag="bg1")
            bout_mu_sb = bpool.tile([4, 1], F32, tag="bout_mu")
            bout_ls_sb = bpool.tile([4, 1], F32, tag="bout_ls")
            bp_sb = bpool.tile([1, 1], F32, tag="bp")
            # ---- state ----
            h0_ab = [state.tile([128, KT, BLOC], F32R, tag="h0a", name="h0a"),
                     state.tile([128, KT, BLOC], F32R, tag="h0b", name="h0b")]
            c0_sb = state.tile([128, KT, BLOC], F32, tag="c0")
            h1_ab = [state.tile([128, KT, BLOC], F32R, tag="h1a", name="h1a"),
                     state.tile([128, KT, BLOC], F32R, tag="h1b", name="h1b")]
            c1_sb = state.tile([128, KT, BLOC], F32, tag="c1")
            cur_sb = state.tile([NIN, BLOC], F32R, tag="cur")

            # ---- initial loads, in consumption order ----
            nc.sync.dma_start(out=weff0_sb[:], in_=weff0[:, :])
            nc.sync.dma_start(out=cur_sb[:], in_=cur0[:, :])
            nc.sync.dma_start(out=bg0_sb[:], in_=bg0[:, :])
            whh0_v = whh0.rearrange("(k p) g -> k p g", p=128)
            h0_v = h0.rearrange("(k p) b -> k p b", p=128)
            c0_v = c0.rearrange("(k p) b -> k p b", p=128)
            for k in range(KT):
                nc.sync.dma_start(out=whh0_sb[:, k, :], in_=whh0_v[k])
                nc.sync.dma_start(out=h0_ab[0][:, k, :], in_=h0_v[k])
                nc.sync.dma_start(out=c0_sb[:, k, :], in_=c0_v[k])
            nc.sync.dma_start(out=bg1_sb[:], in_=bg1[:, :])
            wih1_v = wih1.rearrange("(k p) g -> k p g", p=128)
            whh1_v = whh1.rearrange("(k p) g -> k p g", p=128)
            h1_v = h1.rearrange("(k p) b -> k p b", p=128)
            c1_v = c1.rearrange("(k p) b -> k p b", p=128)
            for k in range(KT):
                nc.sync.dma_start(out=wih1_sb[:, k, :], in_=wih1_v[k])
                nc.sync.dma_start(out=h1_ab[0][:, k, :], in_=h1_v[k])
                nc.sync.dma_start(out=c1_sb[:, k, :], in_=c1_v[k])
            for k in range(KT):
                nc.sync.dma_start(out=whh1_sb[:, k, :], in_=whh1_v[k])
            nc.sync.dma_start(out=wout_sb[:],
                              in_=wout.rearrange("(k p) o -> p k o", p=128))
            nc.sync.dma_start(out=wp_sb[:],
                              in_=wp.rearrange("(k p) o -> p k o", p=128))
            nc.sync.dma_start(out=bout_mu_sb[:], in_=bout[0:4, :])
            nc.sync.dma_start(out=bout_ls_sb[:], in_=bout[4:8, :])
            nc.sync.dma_start(out=bp_sb[:], in_=bp[:, :])

            for d in range(DAYS):
                h0_sb, h0_new = h0_ab[d % 2], h0_ab[(d + 1) % 2]
                h1_sb, h1_new = h1_ab[d % 2], h1_ab[(d + 1) % 2]
                cur_r = cur_sb[:]
                # layer 0: gates = W_eff @ cur + Whh0 @ h0 + b_eff0
                specs0 = [(lambda sl: weff0_sb[:, sl], cur_r)]
                for k in range(KT):
                    specs0.append(
                        (lambda sl, k=k: whh0_sb[:, k, sl],
                         h0_sb[:, k, :]))
                _lstm_layer(nc, pspool, gpool, tpool, specs0,
                            bg0_sb, c0_sb, h0_new)

                # layer 1: gates = Wih1 @ h0 + Whh1 @ h1 + b1
                specs1 = []
                for k in range(KT):
                    specs1.append(
                        (lambda sl, k=k: wih1_sb[:, k, sl],
                         h0_new[:, k, :]))
                for k in range(KT):
                    specs1.append(
                        (lambda sl, k=k: whh1_sb[:, k, sl],
                         h1_sb[:, k, :]))
                _lstm_layer(nc, pspool, gpool, tpool, specs1,
                            bg1_sb, c1_sb, h1_new)

                # heads: proj (mu | log_sigma) and p_rain
                hp = pspool.tile([128, 4, BLOC], F32, tag="ps")
                for k in range(KT):
                    nc.tensor.matmul(hp[0:4, 0, :], wout_sb[:, k, 0:4],
                                     h1_new[:, k, :],
                                     start=(k == 0), stop=(k == KT - 1))
                for k in range(KT):
                    nc.tensor.matmul(hp[0:4, 1, :], wout_sb[:, k, 4:8],
                                     h1_new[:, k, :],
                                     start=(k == 0), stop=(k == KT - 1))
                for k in range(KT):
                    nc.tensor.matmul(hp[0:1, 2, :], wp_sb[:, k, :],
                                     h1_new[:, k, :],
                                     start=(k == 0), stop=(k == KT - 1))
                # mu: also the next day's input (stop_gradient is identity)
                nc.scalar.activation(cur_sb[:], hp[0:4, 0, :], AF.Identity,
                                     bias=bout_mu_sb[:])
                mu = tpool.tile([NTGT, BLOC], F32, tag="mu")
                nc.scalar.activation(mu[:], hp[0:4, 0, :], AF.Identity,
                                     bias=bout_mu_sb[:])
                nc.sync.dma_start(out=mu_out[d], in_=mu[:])
                ls = tpool.tile([NTGT, BLOC], F32, tag="ls")
                nc.scalar.activation(ls[:], hp[0:4, 1, :], AF.Identity,
                                     bias=bout_ls_sb[:])
                nc.vector.tensor_scalar(out=ls[:], in0=ls[:], scalar1=-6.0,
                                        scalar2=2.0, op0=ALU.max, op1=ALU.min)
                sg = tpool.tile([NTGT, BLOC], F32, tag="sg")
                nc.scalar.activation(sg[:], ls[:], AF.Exp)
                nc.sync.dma_start(out=sg_out[d], in_=sg[:])
                pr = tpool.tile([1, BLOC], F32, tag="pr")
                nc.scalar.activation(pr[:], hp[0:1, 2, :], AF.Sigmoid,
                                     bias=bp_sb[:])
                nc.sync.dma_start(out=p_out[d], in_=pr[:])

    nc.compile()
    return nc


def _prep_inputs(inputs):
    """Host-side: transposes, weight folding, per-core sharding."""
    f = {k: np.asarray(v, dtype=np.float32) for k, v in inputs.items()}
    H = HIDDEN

    Wih0 = f["Wih0"].astype(np.float64)
    W_in = f["W_in"].astype(np.float64)
    weff0 = np.ascontiguousarray((Wih0 @ W_in).T).astype(np.float32)  # [4, 2048]
    beff0 = (f["bih0"].astype(np.float64) + f["bhh0"].astype(np.float64)
             + Wih0 @ f["b_in"].astype(np.float64)).astype(np.float32)
    bg0 = np.ascontiguousarray(beff0.reshape(GT, 128).T)
    bg1 = np.ascontiguousarray((f["bih1"] + f["bhh1"]).reshape(GT, 128).T)

    perm = [0, 2, 4, 6, 1, 3, 5, 7]
    wout = np.ascontiguousarray(f["W_out"][perm].T)            # [512, 8]
    bout = np.ascontiguousarray(f["b_out"][perm].reshape(8, 1))
    wp = np.ascontiguousarray(f["W_p"].T)                      # [512, 1]
    bp = np.ascontiguousarray(f["b_p"].reshape(1, 1))

    whh0 = np.ascontiguousarray(f["Whh0"].T)                   # [512, 2048]
    wih1 = np.ascontiguousarray(f["Wih1"].T)
    whh1 = np.ascontiguousarray(f["Whh1"].T)

    cur0 = np.ascontiguousarray(
        np.broadcast_to(f["start_emb"].reshape(NIN, 1), (NIN, BLOC)))

    shared = dict(weff0=weff0, whh0=whh0, wih1=wih1, whh1=whh1,
                  wout=wout, wp=wp, bg0=bg0, bg1=bg1, bout=bout, bp=bp,
                  cur0=cur0)
    in_maps = []
    for i in range(NCORES):
        sl = slice(i * BLOC, (i + 1) * BLOC)
        m = dict(shared)
        m["h0"] = np.ascontiguousarray(f["h"][0, sl].T)
        m["c0"] = np.ascontiguousarray(f["c"][0, sl].T)
        m["h1"] = np.ascontiguousarray(f["h"][1, sl].T)
        m["c1"] = np.ascontiguousarray(f["c"][1, sl].T)
        in_maps.append(m)
    return in_maps


def run(inputs, **spmd_kwargs):
    """Build (cached), run on 8 cores, gather. Returns (results, raw)."""
    if "nc" not in _CACHE:
        _CACHE["nc"] = build_nc()
    nc = _CACHE["nc"]
    in_maps = _prep_inputs(inputs)
    raw = run_bass_kernel_spmd(nc, in_maps, core_ids=list(range(NCORES)),
                               **spmd_kwargs)
    mu = np.empty((BATCH, DAYS, NTGT), np.float32)
    sg = np.empty((BATCH, DAYS, NTGT), np.float32)
    p = np.empty((BATCH, DAYS, 1), np.float32)
    for i, r in enumerate(raw.results):
        sl = slice(i * BLOC, (i + 1) * BLOC)
        mu[sl] = r["mu_out"].transpose(2, 0, 1)
        sg[sl] = r["sg_out"].transpose(2, 0, 1)
        p[sl] = r["p_out"].transpose(2, 0, 1)
    return (mu, sg, p), raw


def kernel(**inputs):
    outs, _ = run(inputs)
    return outs


# revision 10
# speedup vs baseline: 1.0110x; 1.0110x over previous
"""Trainium2 Bass kernel for the 2-layer LSTM decoder (7-day unroll).

Sharding: data-parallel over batch (4096 -> 512 rows/core on 8 cores),
weights replicated. All on-device tensors are stored transposed
([feature, batch]) so no on-device transposes are needed; host numpy
does the (free) transposes and gathers.

Math notes:
  - gates = Wih @ x + Whh @ h + b, with x = W_in @ cur + b_in folded in:
      W_eff = Wih0 @ W_in.T  (2048x4),  b_eff0 = bih0 + bhh0 + Wih0 @ b_in
    so layer0 gates = W_eff @ cur + Whh0 @ h0 + b_eff0 (cur is 4-dim).
  - W_out rows are permuted on host to [mu(4) | log_sigma(4)] so the
    device sees clean partition slices.
  - Matmuls run in float32r (TF32-like, full PE rate at N=512).
"""
import numpy as np

import concourse.bass as bass
import concourse.mybir as mybir
import concourse.tile as tile
from concourse import bacc
from concourse.bass_utils import run_bass_kernel_spmd

HIDDEN = 512
BATCH = 4096
DAYS = 7
NTGT = 4
NIN = 4
NCORES = 8
BLOC = BATCH // NCORES          # 512 batch rows per core
KT = HIDDEN // 128              # 4 contraction tiles
GT = 4 * HIDDEN // 128          # 16 gate tiles
F32 = mybir.dt.float32
F32R = mybir.dt.float32r
import os
if os.environ.get("KMM_DTYPE") == "f32":
    F32R = mybir.dt.float32
AF = mybir.ActivationFunctionType
ALU = mybir.AluOpType

_CACHE = {}


def _lstm_layer(nc, pspool, gpool, tpool, mm_specs, bias_sb, c_sb, h_new_sb):
    """One LSTM cell update in transposed layout.

    mm_specs: list of (lhsT_fn, rhs) pairs; lhsT_fn(gate_slice) -> AP.
    Gates accumulate in PSUM per 128-row chunk j; ScalarE applies
    bias+nonlinearity straight out of PSUM; VectorE does the cell update.
    """
    for j in range(KT):
        ps = pspool.tile([128, 4, BLOC], F32, tag="ps")
        for g in range(4):  # i, f, g, o
            m = g * KT + j
            sl = slice(m * 128, (m + 1) * 128)
            n_mm = len(mm_specs)
            for idx, (lhsT_fn, rhs) in enumerate(mm_specs):
                nc.tensor.matmul(
                    ps[:, g, :], lhsT_fn(sl), rhs,
                    start=(idx == 0), stop=(idx == n_mm - 1),
                )
        gi = gpool.tile([128, BLOC], F32, tag="gi")
        gf = gpool.tile([128, BLOC], F32, tag="gf")
        gg = gpool.tile([128, BLOC], F32, tag="gg")
        go = gpool.tile([128, BLOC], F32, tag="go")
        nc.scalar.activation(gi[:], ps[:, 0, :], AF.Sigmoid,
                             bias=bias_sb[:, 0 * KT + j: 0 * KT + j + 1])
        nc.scalar.activation(gf[:], ps[:, 1, :], AF.Sigmoid,
                             bias=bias_sb[:, 1 * KT + j: 1 * KT + j + 1])
        nc.scalar.activation(gg[:], ps[:, 2, :], AF.Tanh,
                             bias=bias_sb[:, 2 * KT + j: 2 * KT + j + 1])
        nc.scalar.activation(go[:], ps[:, 3, :], AF.Sigmoid,
                             bias=bias_sb[:, 3 * KT + j: 3 * KT + j + 1])
        t1 = tpool.tile([128, BLOC], F32, tag="t1")
        t2 = tpool.tile([128, BLOC], F32, tag="t2")
        th = tpool.tile([128, BLOC], F32, tag="th")
        nc.vector.tensor_mul(t1[:], gi[:], gg[:])
        nc.vector.tensor_mul(t2[:], gf[:], c_sb[:, j, :])
        nc.vector.tensor_add(c_sb[:, j, :], t1[:], t2[:])
        nc.scalar.activation(th[:], c_sb[:, j, :], AF.Tanh)
        nc.vector.tensor_mul(h_new_sb[:, j, :], go[:], th[:])


def build_nc():
    nc = bacc.Bacc("TRN2", target_bir_lowering=False, debug=False,
                   num_devices=NCORES)
    G4 = 4 * HIDDEN

    weff0 = nc.declare_dram_parameter("weff0", [NIN, G4], F32R, isOutput=False)
    whh0 = nc.declare_dram_parameter("whh0", [HIDDEN, G4], F32R, isOutput=False)
    wih1 = nc.declare_dram_parameter("wih1", [HIDDEN, G4], F32R, isOutput=False)
    whh1 = nc.declare_dram_parameter("whh1", [HIDDEN, G4], F32R, isOutput=False)
    wout = nc.declare_dram_parameter("wout", [HIDDEN, 8], F32R, isOutput=False)
    wp = nc.declare_dram_parameter("wp", [HIDDEN, 1], F32R, isOutput=False)
    bg0 = nc.declare_dram_parameter("bg0", [128, GT], F32, isOutput=False)
    bg1 = nc.declare_dram_parameter("bg1", [128, GT], F32, isOutput=False)
    bout = nc.declare_dram_parameter("bout", [8, 1], F32, isOutput=False)
    bp = nc.declare_dram_parameter("bp", [1, 1], F32, isOutput=False)
    cur0 = nc.declare_dram_parameter("cur0", [NIN, BLOC], F32R, isOutput=False)
    h0 = nc.declare_dram_parameter("h0", [HIDDEN, BLOC], F32R, isOutput=False)
    c0 = nc.declare_dram_parameter("c0", [HIDDEN, BLOC], F32, isOutput=False)
    h1 = nc.declare_dram_parameter("h1", [HIDDEN, BLOC], F32R, isOutput=False)
    c1 = nc.declare_dram_parameter("c1", [HIDDEN, BLOC], F32, isOutput=False)
    mu_out = nc.declare_dram_parameter("mu_out", [DAYS, NTGT, BLOC], F32, isOutput=True)
    sg_out = nc.declare_dram_parameter("sg_out", [DAYS, NTGT, BLOC], F32, isOutput=True)
    p_out = nc.declare_dram_parameter("p_out", [DAYS, 1, BLOC], F32, isOutput=True)

    with tile.TileContext(nc) as tc:
        with (
            tc.tile_pool(name="wpool", bufs=1) as wpool,
            tc.tile_pool(name="state", bufs=1) as state,
            tc.tile_pool(name="bias", bufs=1) as bpool,
            tc.tile_pool(name="gpool", bufs=2) as gpool,
            tc.tile_pool(name="tpool", bufs=2) as tpool,
            tc.tile_pool(name="pspool", bufs=2, space="PSUM") as pspool,
        ):
            # ---- resident weights ----
            weff0_sb = wpool.tile([NIN, G4], F32R, tag="weff0")
            whh0_sb = wpool.tile([128, KT, G4], F32R, tag="whh0")
            wih1_sb = wpool.tile([128, KT, G4], F32R, tag="wih1")
            whh1_sb = wpool.tile([128, KT, G4], F32R, tag="whh1")
            wout_sb = wpool.tile([128, KT, 8], F32R, tag="wout")
            wp_sb = wpool.tile([128, KT, 1], F32R, tag="wp")
            bg0_sb = bpool.tile([128, GT], F32, tag="bg0")
            bg1_sb = bpool.tile([128, GT], F32, tag="bg1")
            bout_mu_sb = bpool.tile([4, 1], F32, tag="bout_mu")
            bout_ls_sb = bpool.tile([4, 1], F32, tag="bout_ls")
            bp_sb = bpool.tile([1, 1], F32, tag="bp")
            # ---- state ----
            h0_ab = [state.tile([128, KT, BLOC], F32R, tag="h0a", name="h0a"),
                     state.tile([128, KT, BLOC], F32R, tag="h0b", name="h0b")]
            c0_sb = state.tile([128, KT, BLOC], F32, tag="c0")
            h1_ab = [state.tile([128, KT, BLOC], F32R, tag="h1a", name="h1a"),
                     state.tile([128, KT, BLOC], F32R, tag="h1b", name="h1b")]
            c1_sb = state.tile([128, KT, BLOC], F32, tag="c1")
            cur_sb = state.tile([NIN, BLOC], F32R, tag="cur")

            # ---- initial loads, in consumption order ----
            nc.sync.dma_start(out=weff0_sb[:], in_=weff0[:, :])
            nc.sync.dma_start(out=cur_sb[:], in_=cur0[:, :])
            nc.sync.dma_start(out=bg0_sb[:], in_=bg0[:, :])
            whh0_v = whh0.rearrange("(k p) g -> k p g", p=128)
            h0_v = h0.rearrange("(k p) b -> k p b", p=128)
            c0_v = c0.rearrange("(k p) b -> k p b", p=128)
            for k in range(KT):
                nc.sync.dma_start(out=whh0_sb[:, k, :], in_=whh0_v[k])
                nc.sync.dma_start(out=h0_ab[0][:, k, :], in_=h0_v[k])
                nc.sync.dma_start(out=c0_sb[:, k, :], in_=c0_v[k])
            nc.sync.dma_start(out=bg1_sb[:], in_=bg1[:, :])
            wih1_v = wih1.rearrange("(k p) g -> k p g", p=128)
            whh1_v = whh1.rearrange("(k p) g -> k p g", p=128)
            h1_v = h1.rearrange("(k p) b -> k p b", p=128)
            c1_v = c1.rearrange("(k p) b -> k p b", p=128)
            for k in range(KT):
                nc.sync.dma_start(out=wih1_sb[:, k, :], in_=wih1_v[k])
                nc.sync.dma_start(out=h1_ab[0][:, k, :], in_=h1_v[k])
                nc.sync.dma_start(out=c1_sb[:, k, :], in_=c1_v[k])
            for k in range(KT):
                nc.sync.dma_start(out=whh1_sb[:, k, :], in_=whh1_v[k])
            nc.sync.dma_start(out=wout_sb[:],
                              in_=wout.rearrange("(k p) o -> p k o", p=128))
            nc.sync.dma_start(out=wp_sb[:],
                              in_=wp.rearrange("(k p) o -> p k o", p=128))
            nc.sync.dma_start(out=bout_mu_sb[:], in_=bout[0:4, :])
            nc.sync.dma_start(out=bout_ls_sb[:], in_=bout[4:8, :])
            nc.sync.dma_start(out=bp_sb[:], in_=bp[:, :])

            reps = int(os.environ.get("KREPS", "1"))
            for step in range(reps * DAYS):
                d = step % DAYS
                h0_sb, h0_new = h0_ab[step % 2], h0_ab[(step + 1) % 2]
                h1_sb, h1_new = h1_ab[step % 2], h1_ab[(step + 1) % 2]
                cur_r = cur_sb[:]
                # layer 0: gates = Whh0 @ h0 + W_eff @ cur + b_eff0.
                # Recurrent terms first: h0 (prev day) is ready long before
                # cur (this day's heads), so the PE never waits on cur.
                specs0 = []
                for k in range(KT):
                    specs0.append(
                        (lambda sl, k=k: whh0_sb[:, k, sl],
                         h0_sb[:, k, :]))
                specs0.append((lambda sl: weff0_sb[:, sl], cur_r))
                _lstm_layer(nc, pspool, gpool, tpool, specs0,
                            bg0_sb, c0_sb, h0_new)

                # layer 1: gates = Whh1 @ h1 + Wih1 @ h0_new + b1.
                # Whh1 terms first (h1 is from prev day); Wih1 terms in k
                # order matching h0_new chunk completion order.
                specs1 = []
                for k in range(KT):
                    specs1.append(
                        (lambda sl, k=k: whh1_sb[:, k, sl],
                         h1_sb[:, k, :]))
                for k in range(KT):
                    specs1.append(
                        (lambda sl, k=k: wih1_sb[:, k, sl],
                         h0_new[:, k, :]))
                _lstm_layer(nc, pspool, gpool, tpool, specs1,
                            bg1_sb, c1_sb, h1_new)

                # heads: proj (mu | log_sigma) and p_rain
                hp = pspool.tile([128, 4, BLOC], F32, tag="ps")
                for k in range(KT):
                    nc.tensor.matmul(hp[0:4, 0, :], wout_sb[:, k, 0:4],
                                     h1_new[:, k, :],
                                     start=(k == 0), stop=(k == KT - 1))
                for k in range(KT):
                    nc.tensor.matmul(hp[0:4, 1, :], wout_sb[:, k, 4:8],
                                     h1_new[:, k, :],
                                     start=(k == 0), stop=(k == KT - 1))
                for k in range(KT):
                    nc.tensor.matmul(hp[0:1, 2, :], wp_sb[:, k, :],
                                     h1_new[:, k, :],
                                     start=(k == 0), stop=(k == KT - 1))
                # mu: also the next day's input (stop_gradient is identity)
                nc.scalar.activation(cur_sb[:], hp[0:4, 0, :], AF.Identity,
                                     bias=bout_mu_sb[:])
                mu = tpool.tile([NTGT, BLOC], F32, tag="mu")
                nc.scalar.activation(mu[:], hp[0:4, 0, :], AF.Identity,
                                     bias=bout_mu_sb[:])
                nc.sync.dma_start(out=mu_out[d], in_=mu[:])
                ls = tpool.tile([NTGT, BLOC], F32, tag="ls")
                nc.scalar.activation(ls[:], hp[0:4, 1, :], AF.Identity,
                                     bias=bout_ls_sb[:])
                nc.vector.tensor_scalar(out=ls[:], in0=ls[:], scalar1=-6.0,
                                        scalar2=2.0, op0=ALU.max, op1=ALU.min)
                sg = tpool.tile([NTGT, BLOC], F32, tag="sg")
                nc.scalar.activation(sg[:], ls[:], AF.Exp)
                nc.sync.dma_start(out=sg_out[d], in_=sg[:])
                pr = tpool.tile([1, BLOC], F32, tag="pr")
                nc.scalar.activation(pr[:], hp[0:1, 2, :], AF.Sigmoid,
                                     bias=bp_sb[:])
                nc.sync.dma_start(out=p_out[d], in_=pr[:])

    nc.compile()
    return nc


def _prep_inputs(inputs):
    """Host-side: transposes, weight folding, per-core sharding."""
    f = {k: np.asarray(v, dtype=np.float32) for k, v in inputs.items()}
    H = HIDDEN

    Wih0 = f["Wih0"].astype(np.float64)
    W_in = f["W_in"].astype(np.float64)
    weff0 = np.ascontiguousarray((Wih0 @ W_in).T).astype(np.float32)  # [4, 2048]
    beff0 = (f["bih0"].astype(np.float64) + f["bhh0"].astype(np.float64)
             + Wih0 @ f["b_in"].astype(np.float64)).astype(np.float32)
    bg0 = np.ascontiguousarray(beff0.reshape(GT, 128).T)
    bg1 = np.ascontiguousarray((f["bih1"] + f["bhh1"]).reshape(GT, 128).T)

    perm = [0, 2, 4, 6, 1, 3, 5, 7]
    wout = np.ascontiguousarray(f["W_out"][perm].T)            # [512, 8]
    bout = np.ascontiguousarray(f["b_out"][perm].reshape(8, 1))
    wp = np.ascontiguousarray(f["W_p"].T)                      # [512, 1]
    bp = np.ascontiguousarray(f["b_p"].reshape(1, 1))

    whh0 = np.ascontiguousarray(f["Whh0"].T)                   # [512, 2048]
    wih1 = np.ascontiguousarray(f["Wih1"].T)
    whh1 = np.ascontiguousarray(f["Whh1"].T)

    cur0 = np.ascontiguousarray(
        np.broadcast_to(f["start_emb"].reshape(NIN, 1), (NIN, BLOC)))

    shared = dict(weff0=weff0, whh0=whh0, wih1=wih1, whh1=whh1,
                  wout=wout, wp=wp, bg0=bg0, bg1=bg1, bout=bout, bp=bp,
                  cur0=cur0)
    in_maps = []
    for i in range(NCORES):
        sl = slice(i * BLOC, (i + 1) * BLOC)
        m = dict(shared)
        m["h0"] = np.ascontiguousarray(f["h"][0, sl].T)
        m["c0"] = np.ascontiguousarray(f["c"][0, sl].T)
        m["h1"] = np.ascontiguousarray(f["h"][1, sl].T)
        m["c1"] = np.ascontiguousarray(f["c"][1, sl].T)
        in_maps.append(m)
    return in_maps


def run(inputs, **spmd_kwargs):
    """Build (cached), run on 8 cores, gather. Returns (results, raw)."""
    if "nc" not in _CACHE:
        _CACHE["nc"] = build_nc()
    nc = _CACHE["nc"]
    in_maps = _prep_inputs(inputs)
    raw = run_bass_kernel_spmd(nc, in_maps, core_ids=list(range(NCORES)),
                               **spmd_kwargs)
    mu = np.empty((BATCH, DAYS, NTGT), np.float32)
    sg = np.empty((BATCH, DAYS, NTGT), np.float32)
    p = np.empty((BATCH, DAYS, 1), np.float32)
    for i, r in enumerate(raw.results):
        sl = slice(i * BLOC, (i + 1) * BLOC)
        mu[sl] = r["mu_out"].transpose(2, 0, 1)
        sg[sl] = r["sg_out"].transpose(2, 0, 1)
        p[sl] = r["p_out"].transpose(2, 0, 1)
    return (mu, sg, p), raw


def kernel(**inputs):
    outs, _ = run(inputs)
    return outs
